# revision 16
# baseline (speedup 1.0000x reference)
"""Causal attention kernel for Trainium2 (Bass/Tile), 8-core SPMD.

Problem: B=16, S=2048, D=128 fp32 causal attention
    scores = Q @ K^T            (per batch)
    scores -= INF * triu(k=1)   (before scaling, as in reference)
    attn = softmax(scores / sqrt(D))
    out = attn @ V

Sharding: batch dim across 8 cores, 2 batches per core, no communication.

Per-core dataflow (per batch, per 512-wide q-block, per 128-wide k-chunk):
    S^T[k, q] = (K^T chunk).T @ Q^T slice      (contract d on partitions)
    diag chunks: += triangular -1e9 mask (DVE)
    P^T = exp(S^T * 1/sqrt(D))                 (ACT, PSUM -> SBUF fp32r)
    O^T[d, q] += V_chunk.T @ P^T chunk          (PSUM accumulate)
    l[q]      += allones.T @ P^T chunk          (rowsum, broadcast on all rows)
    out[q, d] = transpose(O^T * (1/l))          (DVE scale + PE transpose)

All matmuls run in fp32r (full-rate fp32 on the PE; producers round).
Evacuation of q-block N is deferred past q-block N+1's first group so the
in-order PE queue never stalls on the DVE normalize chain.
"""

import os

os.environ.setdefault("MYCRO_LOCAL_CACHE", "1")

import math

import numpy as np

import concourse.bass as bass
import concourse.mybir as mybir
import concourse.tile as tile
from concourse import bacc
from concourse.bass_utils import run_bass_kernel_spmd
from concourse.masks import make_identity

F32 = mybir.dt.float32
F32R = mybir.dt.float32r
EXPF = mybir.ActivationFunctionType.Exp

N_CORES = 8
B = 16
S = 2048
D = 128
BPC = B // N_CORES  # batches per core
SCALE = 1.0 / math.sqrt(float(D))
NEG = -1.0e9
NQB = S // 512  # q blocks per batch
NCH = S // 128  # k chunks per batch


def build():
    nc = bacc.Bacc("TRN2", target_bir_lowering=False, debug=False, num_devices=N_CORES)
    q_d = nc.dram_tensor("qt", [BPC, D, S], F32, kind="ExternalInput")
    k_d = nc.dram_tensor("kt", [BPC, D, S], F32, kind="ExternalInput")
    v_d = nc.dram_tensor("v", [BPC, S, D], F32, kind="ExternalInput")
    o_d = nc.dram_tensor("o", [BPC, D, S], F32, kind="ExternalOutput")

    with tile.TileContext(nc) as tc:
        with (
            tc.tile_pool(name="const", bufs=1) as constp,
            tc.tile_pool(name="tpose", bufs=2) as tposep,
            tc.tile_pool(name="pt", bufs=4) as ptp,
            tc.tile_pool(name="evac", bufs=2) as evacp,
            tc.tile_pool(name="stps", bufs=2, space="PSUM") as stps,
            tc.tile_pool(name="otps", bufs=2, space="PSUM") as otps,
            tc.tile_pool(name="lps", bufs=2, space="PSUM") as lps,
        ):
            # ---- constants ----
            ident32 = constp.tile([128, 128], F32, name="ident32")
            make_identity(nc, ident32[:])
            zb = constp.tile([128, 1], F32, name="zb")
            nc.gpsimd.memset(zb[:], 0.0)
            ones_f = constp.tile([128, 128], F32, name="ones_f")
            nc.gpsimd.memset(ones_f[:], 1.0)
            ones_r = constp.tile([128, 128], F32R, name="ones_r")
            nc.vector.tensor_copy(ones_r[:], ones_f[:])
            identR = constp.tile([128, 128], F32R, name="identR")
            nc.vector.tensor_copy(identR[:], ident32[:])
            # u01[i, k] = 1 iff i < k; with rhs=-1e9*I the product writes the
            # causal mask NEG*[q < k] straight into PSUM on the PE.
            u01_f = constp.tile([128, 128], F32, name="u01_f")
            nc.gpsimd.memset(u01_f[:], 1.0)
            nc.gpsimd.affine_select(
                out=u01_f[:],
                in_=u01_f[:],
                compare_op=mybir.AluOpType.is_ge,
                fill=0.0,
                base=-1,
                pattern=[[1, 128]],
                channel_multiplier=-1,
            )
            u01_r = constp.tile([128, 128], F32R, name="u01_r")
            nc.vector.tensor_copy(u01_r[:], u01_f[:])
            idneg_r = constp.tile([128, 128], F32R, name="idneg_r")
            with nc.allow_low_precision("f32r is full-width fp32 storage"):
                nc.vector.tensor_scalar_mul(idneg_r[:], ident32[:], NEG)
            idneg2_r = constp.tile([128, 384], F32R, name="idneg2_r")
            nc.gpsimd.memset(idneg2_r[:].bitcast(F32), 0.0)
            with nc.allow_low_precision("f32r is full-width fp32 storage"):
                nc.vector.tensor_scalar_mul(
                    idneg2_r[:, 0:128], ident32[:], NEG
                )
                nc.vector.tensor_scalar_mul(
                    idneg2_r[:, 256:384], ident32[:], NEG
                )

            # HAM warmup: dense PE activity while the first DMAs land
            warm_ps = stps.tile([128, 128], F32, name="warm_ps", tag="stps")
            for _ in range(30):
                nc.tensor.matmul(
                    warm_ps[:], identR[:], identR[:], start=True, stop=True
                )

            # software-pipeline state: PV/rowsum of group g is emitted
            # after S+exp of group g+1 (PE never queues behind exp); the
            # transpose/store tail of q-block N is emitted during N+1.
            pending_pv = [None]
            pending_evac = [None]

            def flush_pv():
                if pending_pv[0] is not None:
                    pending_pv[0]()
                    pending_pv[0] = None

            def flush_evac():
                if pending_evac[0] is not None:
                    pending_evac[0]()
                    pending_evac[0] = None

            for b in range(BPC):
                # ---- load Q^T, K^T (host pre-transposed), V via cast DMA ----
                qt = tposep.tile([128, S], F32R, name="qt")
                kt = tposep.tile([128, S], F32R, name="kt")
                vr = tposep.tile([128, S], F32R, name="vr")
                for p4 in range(4):
                    sl = slice(p4 * 512, (p4 + 1) * 512)
                    nc.sync.dma_start(qt[:, sl], q_d[b, :, sl].bitcast(F32R))
                    nc.sync.dma_start(kt[:, sl], k_d[b, :, sl].bitcast(F32R))
                # vr[:, j*128 + d] = V[b, j*128 + p, d]
                nc.sync.dma_start(
                    vr[:].rearrange("p (j d) -> p j d", d=128),
                    v_d[b].rearrange("(j p) d -> p j d", p=128).bitcast(F32R),
                )

                # ---- q blocks ----
                for qb in range(NQB):
                    n_full = 4 * qb
                    n_ch = n_full + 4
                    q0 = qb * 512

                    # (chunks, extent, is_diag); st tiles are [128, 1024]
                    groups = []
                    jf = 0
                    while jf < n_full:
                        g = min(2, n_full - jf)
                        groups.append(
                            (
                                [(jf + c, 0, 512, c * 512) for c in range(g)],
                                g * 512,
                                False,
                            )
                        )
                        jf += g
                    groups.append(
                        ([(n_full, 0, 512, 0), (n_full + 1, 128, 384, 512)], 896, True)
                    )
                    groups.append(
                        (
                            [(n_full + 2, 256, 256, 0), (n_full + 3, 384, 128, 256)],
                            384,
                            True,
                        )
                    )

                    ot = otps.tile([128, 512], F32, name="ot")
                    lp = lps.tile([128, 512], F32, name="lp", tag="lp")

                    for gi, (chunks, extent, is_diag) in enumerate(groups):
                        st = stps.tile([128, 1024], F32, name="st", tag="stps")
                        if is_diag and chunks[0][3] == 0 and len(chunks) == 2 and chunks[1][3] == 256:
                            # diag B: one mask matmul covers both chunks
                            nc.tensor.matmul(
                                st[:, 0:384],
                                u01_r[:],
                                idneg2_r[:],
                                start=True,
                                stop=False,
                            )
                            premasked = True
                        else:
                            premasked = False
                        for (j, qoff, width, col) in chunks:
                            if is_diag and not premasked:
                                # write NEG*[q<k] into the first 128 cols,
                                # then accumulate the scores on top
                                nc.tensor.matmul(
                                    st[:, col : col + 128],
                                    u01_r[:],
                                    idneg_r[:],
                                    start=True,
                                    stop=False,
                                )
                            nc.tensor.matmul(
                                st[:, col : col + width],
                                kt[:, j * 128 : (j + 1) * 128],
                                qt[:, q0 + qoff : q0 + qoff + width],
                                start=not is_diag,
                                stop=True,
                            )
                        pt = ptp.tile([128, 1024], F32R, name="pt", tag="pt")
                        nc.scalar.activation(
                            pt[:, 0:extent],
                            st[:, 0:extent],
                            EXPF,
                            bias=zb[:],
                            scale=SCALE,
                        )
                        flush_pv()
                        if gi == 1:
                            flush_evac()

                        def pv(
                            chunks=chunks,
                            ot=ot,
                            lp=lp,
                            pt=pt,
                            vr=vr,
                            n_ch=n_ch,
                            is_last=(gi == len(groups) - 1),
                            b=b,
                            q0=q0,
                        ):
                            for (j, qoff, width, col) in chunks:
                                nc.tensor.matmul(
                                    ot[:, qoff : qoff + width],
                                    vr[:, j * 128 : (j + 1) * 128],
                                    pt[:, col : col + width],
                                    start=(j == 0),
                                    stop=(j == n_ch - 1),
                                )
                                nc.tensor.matmul(
                                    lp[:, qoff : qoff + width],
                                    ones_r[:],
                                    pt[:, col : col + width],
                                    start=(j == 0),
                                    stop=(j == n_ch - 1),
                                )
                            if not is_last:
                                return
                            # ---- evacuation (DVE): O^T * (1/l) ----
                            # lp rows are all equal (all-ones stationary).
                            # Output stays in [d, q] layout; the host gather
                            # transposes back to [q, d].
                            recip = evacp.tile([128, 512], F32, name="recip")
                            nc.vector.reciprocal(recip[:], lp[:])
                            ots = evacp.tile([128, 512], F32, name="ots")
                            nc.vector.tensor_mul(ots[:], ot[:], recip[:])

                            def evac(b=b, q0=q0, ots=ots):
                                nc.sync.dma_start(
                                    o_d[b, :, q0 : q0 + 512], ots[:]
                                )

                            pending_evac[0] = evac

                        pending_pv[0] = pv

            flush_pv()
            flush_evac()
    nc.compile()
    return nc


_NC_CACHE = None


def _get_nc():
    global _NC_CACHE
    if _NC_CACHE is None:
        _NC_CACHE = build()
    return _NC_CACHE


def kernel(query, key, value, _trace=False):
    nc = _get_nc()
    in_maps = []
    for c in range(N_CORES):
        sl = slice(c * BPC, (c + 1) * BPC)
        in_maps.append(
            {
                "qt": np.ascontiguousarray(
                    np.asarray(query[sl], dtype=np.float32).transpose(0, 2, 1)
                ),
                "kt": np.ascontiguousarray(
                    np.asarray(key[sl], dtype=np.float32).transpose(0, 2, 1)
                ),
                "v": np.ascontiguousarray(value[sl], dtype=np.float32),
            }
        )
    res = run_bass_kernel_spmd(
        nc, in_maps, core_ids=list(range(N_CORES)), trace=_trace
    )
    out = np.concatenate(
        [res.results[c]["o"].transpose(0, 2, 1) for c in range(N_CORES)], axis=0
    )
    out = np.ascontiguousarray(out)
    if _trace:
        return out, res
    return out


# revision 17
# speedup vs baseline: 1.0174x; 1.0174x over previous
"""Causal attention kernel for Trainium2 (Bass/Tile), 8-core SPMD.

Problem: B=16, S=2048, D=128 fp32 causal attention
    scores = Q @ K^T            (per batch)
    scores -= INF * triu(k=1)   (before scaling, as in reference)
    attn = softmax(scores / sqrt(D))
    out = attn @ V

Sharding: batch dim across 8 cores, 2 batches per core, no communication.

Per-core dataflow (per batch, per 512-wide q-block, per 128-wide k-chunk):
    S^T[k, q] = (K^T chunk).T @ Q^T slice      (contract d on partitions)
    diag chunks: += triangular -1e9 mask (DVE)
    P^T = exp(S^T * 1/sqrt(D))                 (ACT, PSUM -> SBUF fp32r)
    O^T[d, q] += V_chunk.T @ P^T chunk          (PSUM accumulate)
    l[q]      += allones.T @ P^T chunk          (rowsum, broadcast on all rows)
    out[q, d] = transpose(O^T * (1/l))          (DVE scale + PE transpose)

All matmuls run in fp32r (full-rate fp32 on the PE; producers round).
Evacuation of q-block N is deferred past q-block N+1's first group so the
in-order PE queue never stalls on the DVE normalize chain.
"""

import os

os.environ.setdefault("MYCRO_LOCAL_CACHE", "1")

import math

import numpy as np

import concourse.bass as bass
import concourse.mybir as mybir
import concourse.tile as tile
from concourse import bacc
from concourse.bass_utils import run_bass_kernel_spmd
from concourse.masks import make_identity

F32 = mybir.dt.float32
F32R = mybir.dt.float32r
EXPF = mybir.ActivationFunctionType.Exp

N_CORES = 8
B = 16
S = 2048
D = 128
BPC = B // N_CORES  # batches per core
SCALE = 1.0 / math.sqrt(float(D))
NEG = -1.0e9
NQB = S // 512  # q blocks per batch
NCH = S // 128  # k chunks per batch


def build():
    nc = bacc.Bacc("TRN2", target_bir_lowering=False, debug=False, num_devices=N_CORES)
    q_d = nc.dram_tensor("qt", [BPC, D, S], F32, kind="ExternalInput")
    k_d = nc.dram_tensor("kt", [BPC, D, S], F32, kind="ExternalInput")
    v_d = nc.dram_tensor("v", [BPC, S, D], F32, kind="ExternalInput")
    o_d = nc.dram_tensor("o", [BPC, D, S], F32, kind="ExternalOutput")

    with tile.TileContext(nc) as tc:
        with (
            tc.tile_pool(name="const", bufs=1) as constp,
            tc.tile_pool(name="tpose", bufs=2) as tposep,
            tc.tile_pool(name="pt", bufs=4) as ptp,
            tc.tile_pool(name="evac", bufs=2) as evacp,
            tc.tile_pool(name="stps", bufs=2, space="PSUM") as stps,
            tc.tile_pool(name="otps", bufs=2, space="PSUM") as otps,
            tc.tile_pool(name="lps", bufs=2, space="PSUM") as lps,
        ):
            # ---- constants ----
            ident32 = constp.tile([128, 128], F32, name="ident32")
            make_identity(nc, ident32[:])
            zb = constp.tile([128, 1], F32, name="zb")
            nc.gpsimd.memset(zb[:], 0.0)
            ones_f = constp.tile([128, 128], F32, name="ones_f")
            nc.gpsimd.memset(ones_f[:], 1.0)
            ones_r = constp.tile([128, 128], F32R, name="ones_r")
            nc.vector.tensor_copy(ones_r[:], ones_f[:])
            identR = constp.tile([128, 128], F32R, name="identR")
            nc.vector.tensor_copy(identR[:], ident32[:])
            # u01[i, k] = 1 iff i < k; with rhs=-1e9*I the product writes the
            # causal mask NEG*[q < k] straight into PSUM on the PE.
            u01_f = constp.tile([128, 128], F32, name="u01_f")
            nc.gpsimd.memset(u01_f[:], 1.0)
            nc.gpsimd.affine_select(
                out=u01_f[:],
                in_=u01_f[:],
                compare_op=mybir.AluOpType.is_ge,
                fill=0.0,
                base=-1,
                pattern=[[1, 128]],
                channel_multiplier=-1,
            )
            u01_r = constp.tile([128, 128], F32R, name="u01_r")
            nc.vector.tensor_copy(u01_r[:], u01_f[:])
            idneg_r = constp.tile([128, 128], F32R, name="idneg_r")
            with nc.allow_low_precision("f32r is full-width fp32 storage"):
                nc.vector.tensor_scalar_mul(idneg_r[:], ident32[:], NEG)
            idneg2_r = constp.tile([128, 384], F32R, name="idneg2_r")
            nc.gpsimd.memset(idneg2_r[:].bitcast(F32), 0.0)
            with nc.allow_low_precision("f32r is full-width fp32 storage"):
                nc.vector.tensor_scalar_mul(
                    idneg2_r[:, 0:128], ident32[:], NEG
                )
                nc.vector.tensor_scalar_mul(
                    idneg2_r[:, 256:384], ident32[:], NEG
                )

            # HAM warmup: dense PE activity while the first DMAs land
            warm_ps = stps.tile([128, 128], F32, name="warm_ps", tag="stps")
            for _ in range(30):
                nc.tensor.matmul(
                    warm_ps[:], identR[:], identR[:], start=True, stop=True
                )

            # software-pipeline state: PV/rowsum of group g is emitted
            # after S+exp of group g+1 (PE never queues behind exp); the
            # transpose/store tail of q-block N is emitted during N+1.
            pending_pv = [None]
            pending_evac = [None]

            def flush_pv():
                if pending_pv[0] is not None:
                    pending_pv[0]()
                    pending_pv[0] = None

            def flush_evac():
                if pending_evac[0] is not None:
                    pending_evac[0]()
                    pending_evac[0] = None

            for b in range(BPC):
                # ---- load Q^T, K^T (host pre-transposed), V via cast DMA ----
                qt = tposep.tile([128, S], F32R, name="qt")
                kt = tposep.tile([128, S], F32R, name="kt")
                vr = tposep.tile([128, S], F32R, name="vr")
                nc.sync.dma_start(qt[:, 0:1024], q_d[b, :, 0:1024].bitcast(F32R))
                nc.sync.dma_start(qt[:, 1024:2048], q_d[b, :, 1024:2048].bitcast(F32R))
                nc.sync.dma_start(kt[:, 0:1024], k_d[b, :, 0:1024].bitcast(F32R))
                nc.sync.dma_start(kt[:, 1024:2048], k_d[b, :, 1024:2048].bitcast(F32R))
                # vr[:, j*128 + d] = V[b, j*128 + p, d]
                nc.sync.dma_start(
                    vr[:].rearrange("p (j d) -> p j d", d=128),
                    v_d[b].rearrange("(j p) d -> p j d", p=128).bitcast(F32R),
                )

                # ---- q blocks ----
                for qb in range(NQB):
                    n_full = 4 * qb
                    n_ch = n_full + 4
                    q0 = qb * 512

                    # (chunks, extent, is_diag); st tiles are [128, 1024]
                    groups = []
                    jf = 0
                    while jf < n_full:
                        g = min(2, n_full - jf)
                        groups.append(
                            (
                                [(jf + c, 0, 512, c * 512) for c in range(g)],
                                g * 512,
                                False,
                            )
                        )
                        jf += g
                    groups.append(
                        ([(n_full, 0, 512, 0), (n_full + 1, 128, 384, 512)], 896, True)
                    )
                    groups.append(
                        (
                            [(n_full + 2, 256, 256, 0), (n_full + 3, 384, 128, 256)],
                            384,
                            True,
                        )
                    )

                    ot = otps.tile([128, 512], F32, name="ot")
                    lp = lps.tile([128, 512], F32, name="lp", tag="lp")

                    for gi, (chunks, extent, is_diag) in enumerate(groups):
                        st = stps.tile([128, 1024], F32, name="st", tag="stps")
                        if is_diag and chunks[0][3] == 0 and len(chunks) == 2 and chunks[1][3] == 256:
                            # diag B: one mask matmul covers both chunks
                            nc.tensor.matmul(
                                st[:, 0:384],
                                u01_r[:],
                                idneg2_r[:],
                                start=True,
                                stop=False,
                            )
                            premasked = True
                        else:
                            premasked = False
                        for (j, qoff, width, col) in chunks:
                            if is_diag and not premasked:
                                # write NEG*[q<k] into the first 128 cols,
                                # then accumulate the scores on top
                                nc.tensor.matmul(
                                    st[:, col : col + 128],
                                    u01_r[:],
                                    idneg_r[:],
                                    start=True,
                                    stop=False,
                                )
                            nc.tensor.matmul(
                                st[:, col : col + width],
                                kt[:, j * 128 : (j + 1) * 128],
                                qt[:, q0 + qoff : q0 + qoff + width],
                                start=not is_diag,
                                stop=True,
                            )
                        pt = ptp.tile([128, 1024], F32R, name="pt", tag="pt")
                        nc.scalar.activation(
                            pt[:, 0:extent],
                            st[:, 0:extent],
                            EXPF,
                            bias=zb[:],
                            scale=SCALE,
                        )
                        flush_pv()
                        if gi == 1:
                            flush_evac()

                        def pv(
                            chunks=chunks,
                            ot=ot,
                            lp=lp,
                            pt=pt,
                            vr=vr,
                            n_ch=n_ch,
                            is_last=(gi == len(groups) - 1),
                            b=b,
                            q0=q0,
                        ):
                            for (j, qoff, width, col) in chunks:
                                nc.tensor.matmul(
                                    ot[:, qoff : qoff + width],
                                    vr[:, j * 128 : (j + 1) * 128],
                                    pt[:, col : col + width],
                                    start=(j == 0),
                                    stop=(j == n_ch - 1),
                                )
                                nc.tensor.matmul(
                                    lp[:, qoff : qoff + width],
                                    ones_r[:],
                                    pt[:, col : col + width],
                                    start=(j == 0),
                                    stop=(j == n_ch - 1),
                                )
                            if not is_last:
                                return
                            # ---- evacuation (DVE): O^T * (1/l) ----
                            # lp rows are all equal (all-ones stationary).
                            # Output stays in [d, q] layout; the host gather
                            # transposes back to [q, d].
                            recip = evacp.tile([128, 512], F32, name="recip")
                            nc.vector.reciprocal(recip[:], lp[:])
                            ots = evacp.tile([128, 512], F32, name="ots")
                            nc.vector.tensor_mul(ots[:], ot[:], recip[:])

                            def evac(b=b, q0=q0, ots=ots):
                                nc.sync.dma_start(
                                    o_d[b, :, q0 : q0 + 512], ots[:]
                                )

                            pending_evac[0] = evac

                        pending_pv[0] = pv

            flush_pv()
            flush_evac()
    nc.compile()
    return nc


_NC_CACHE = None


def _get_nc():
    global _NC_CACHE
    if _NC_CACHE is None:
        _NC_CACHE = build()
    return _NC_CACHE


def kernel(query, key, value, _trace=False):
    nc = _get_nc()
    in_maps = []
    for c in range(N_CORES):
        sl = slice(c * BPC, (c + 1) * BPC)
        in_maps.append(
            {
                "qt": np.ascontiguousarray(
                    np.asarray(query[sl], dtype=np.float32).transpose(0, 2, 1)
                ),
                "kt": np.ascontiguousarray(
                    np.asarray(key[sl], dtype=np.float32).transpose(0, 2, 1)
                ),
                "v": np.ascontiguousarray(value[sl], dtype=np.float32),
            }
        )
    res = run_bass_kernel_spmd(
        nc, in_maps, core_ids=list(range(N_CORES)), trace=_trace
    )
    out = np.concatenate(
        [res.results[c]["o"].transpose(0, 2, 1) for c in range(N_CORES)], axis=0
    )
    out = np.ascontiguousarray(out)
    if _trace:
        return out, res
    return out


# revision 19
# speedup vs baseline: 1.0211x; 1.0036x over previous
"""Causal attention kernel for Trainium2 (Bass/Tile), 8-core SPMD.

Problem: B=16, S=2048, D=128 fp32 causal attention
    scores = Q @ K^T            (per batch)
    scores -= INF * triu(k=1)   (before scaling, as in reference)
    attn = softmax(scores / sqrt(D))
    out = attn @ V

Sharding: batch dim across 8 cores, 2 batches per core, no communication.

Per-core dataflow (per batch, per 512-wide q-block, per 128-wide k-chunk):
    S^T[k, q] = (K^T chunk).T @ Q^T slice      (contract d on partitions)
    diag chunks: += triangular -1e9 mask (DVE)
    P^T = exp(S^T * 1/sqrt(D))                 (ACT, PSUM -> SBUF fp32r)
    O^T[d, q] += V_chunk.T @ P^T chunk          (PSUM accumulate)
    l[q]      += allones.T @ P^T chunk          (rowsum, broadcast on all rows)
    out[q, d] = transpose(O^T * (1/l))          (DVE scale + PE transpose)

All matmuls run in fp32r (full-rate fp32 on the PE; producers round).
Evacuation of q-block N is deferred past q-block N+1's first group so the
in-order PE queue never stalls on the DVE normalize chain.
"""

import os

os.environ.setdefault("MYCRO_LOCAL_CACHE", "1")

import math

import numpy as np

import concourse.bass as bass
import concourse.mybir as mybir
import concourse.tile as tile
from concourse import bacc
from concourse.bass_utils import run_bass_kernel_spmd
from concourse.masks import make_identity

F32 = mybir.dt.float32
F32R = mybir.dt.float32r
EXPF = mybir.ActivationFunctionType.Exp

N_CORES = 8
B = 16
S = 2048
D = 128
BPC = B // N_CORES  # batches per core
SCALE = 1.0 / math.sqrt(float(D))
NEG = -1.0e9
NQB = S // 512  # q blocks per batch
NCH = S // 128  # k chunks per batch


def build():
    nc = bacc.Bacc("TRN2", target_bir_lowering=False, debug=False, num_devices=N_CORES)
    q_d = nc.dram_tensor("qt", [BPC, D, S], F32, kind="ExternalInput")
    k_d = nc.dram_tensor("kt", [BPC, D, S], F32, kind="ExternalInput")
    v_d = nc.dram_tensor("v", [BPC, S, D], F32, kind="ExternalInput")
    o_d = nc.dram_tensor("o", [BPC, D, S], F32, kind="ExternalOutput")

    with tile.TileContext(nc) as tc:
        with (
            tc.tile_pool(name="const", bufs=1) as constp,
            tc.tile_pool(name="tpose", bufs=2) as tposep,
            tc.tile_pool(name="pt", bufs=4) as ptp,
            tc.tile_pool(name="evac", bufs=2) as evacp,
            tc.tile_pool(name="stps", bufs=2, space="PSUM") as stps,
            tc.tile_pool(name="otps", bufs=2, space="PSUM") as otps,
            tc.tile_pool(name="lps", bufs=2, space="PSUM") as lps,
        ):
            # ---- constants ----
            ident32 = constp.tile([128, 128], F32, name="ident32")
            make_identity(nc, ident32[:])
            zb = constp.tile([128, 1], F32, name="zb")
            nc.gpsimd.memset(zb[:], 0.0)
            ones_f = constp.tile([128, 128], F32, name="ones_f")
            nc.gpsimd.memset(ones_f[:], 1.0)
            ones_r = constp.tile([128, 128], F32R, name="ones_r")
            nc.vector.tensor_copy(ones_r[:], ones_f[:])
            identR = constp.tile([128, 128], F32R, name="identR")
            nc.vector.tensor_copy(identR[:], ident32[:])
            zeros_r = constp.tile([128, 1], F32R, name="zeros_r")
            nc.gpsimd.memset(zeros_r[:].bitcast(F32), 0.0)
            sel_f = constp.tile([128, 128], F32, name="sel_f")
            nc.gpsimd.memset(sel_f[:], 0.0)
            for s4 in range(4):
                nc.gpsimd.memset(sel_f[32 * s4 : 32 * s4 + 1, :], 1.0)
            sel_r = constp.tile([128, 128], F32R, name="sel_r")
            nc.vector.tensor_copy(sel_r[:], sel_f[:])
            # u01[i, k] = 1 iff i < k; with rhs=-1e9*I the product writes the
            # causal mask NEG*[q < k] straight into PSUM on the PE.
            u01_f = constp.tile([128, 128], F32, name="u01_f")
            nc.gpsimd.memset(u01_f[:], 1.0)
            nc.gpsimd.affine_select(
                out=u01_f[:],
                in_=u01_f[:],
                compare_op=mybir.AluOpType.is_ge,
                fill=0.0,
                base=-1,
                pattern=[[1, 128]],
                channel_multiplier=-1,
            )
            u01_r = constp.tile([128, 128], F32R, name="u01_r")
            nc.vector.tensor_copy(u01_r[:], u01_f[:])
            idneg_r = constp.tile([128, 128], F32R, name="idneg_r")
            with nc.allow_low_precision("f32r is full-width fp32 storage"):
                nc.vector.tensor_scalar_mul(idneg_r[:], ident32[:], NEG)
            idneg2_r = constp.tile([128, 384], F32R, name="idneg2_r")
            nc.gpsimd.memset(idneg2_r[:].bitcast(F32), 0.0)
            with nc.allow_low_precision("f32r is full-width fp32 storage"):
                nc.vector.tensor_scalar_mul(
                    idneg2_r[:, 0:128], ident32[:], NEG
                )
                nc.vector.tensor_scalar_mul(
                    idneg2_r[:, 256:384], ident32[:], NEG
                )

            # HAM warmup: dense PE activity while the first DMAs land
            warm_ps = stps.tile([128, 128], F32, name="warm_ps", tag="stps")
            for _ in range(30):
                nc.tensor.matmul(
                    warm_ps[:], identR[:], identR[:], start=True, stop=True
                )

            # software-pipeline state: PV/rowsum of group g is emitted
            # after S+exp of group g+1 (PE never queues behind exp); the
            # transpose/store tail of q-block N is emitted during N+1.
            pending_pv = [None]
            pending_evac = [None]

            def flush_pv():
                if pending_pv[0] is not None:
                    pending_pv[0]()
                    pending_pv[0] = None

            def flush_evac():
                if pending_evac[0] is not None:
                    pending_evac[0]()
                    pending_evac[0] = None

            for b in range(BPC):
                # ---- load Q^T, K^T (host pre-transposed), V via cast DMA ----
                qt = tposep.tile([128, S], F32R, name="qt")
                kt = tposep.tile([128, S], F32R, name="kt")
                vr = tposep.tile([128, S], F32R, name="vr")
                nc.sync.dma_start(qt[:, 0:1024], q_d[b, :, 0:1024].bitcast(F32R))
                nc.sync.dma_start(qt[:, 1024:2048], q_d[b, :, 1024:2048].bitcast(F32R))
                nc.sync.dma_start(kt[:, 0:1024], k_d[b, :, 0:1024].bitcast(F32R))
                nc.sync.dma_start(kt[:, 1024:2048], k_d[b, :, 1024:2048].bitcast(F32R))
                # vr[:, j*128 + d] = V[b, j*128 + p, d]
                nc.sync.dma_start(
                    vr[:].rearrange("p (j d) -> p j d", d=128),
                    v_d[b].rearrange("(j p) d -> p j d", p=128).bitcast(F32R),
                )

                # ---- q blocks ----
                for qb in range(NQB):
                    n_full = 4 * qb
                    n_ch = n_full + 4
                    q0 = qb * 512

                    # (chunks, extent, is_diag); st tiles are [128, 1024]
                    groups = []
                    jf = 0
                    while jf < n_full:
                        g = min(2, n_full - jf)
                        groups.append(
                            (
                                [(jf + c, 0, 512, c * 512) for c in range(g)],
                                g * 512,
                                False,
                            )
                        )
                        jf += g
                    groups.append(
                        ([(n_full, 0, 512, 0), (n_full + 1, 128, 384, 512)], 896, True)
                    )
                    groups.append(
                        (
                            [(n_full + 2, 256, 256, 0), (n_full + 3, 384, 128, 256)],
                            384,
                            True,
                        )
                    )

                    ot = otps.tile([128, 512], F32, name="ot")
                    lp = lps.tile([128, 512], F32, name="lp", tag="lp")

                    for gi, (chunks, extent, is_diag) in enumerate(groups):
                        st = stps.tile([128, 1024], F32, name="st", tag="stps")
                        if is_diag and chunks[0][3] == 0 and len(chunks) == 2 and chunks[1][3] == 256:
                            # diag B: one mask matmul covers both chunks
                            nc.tensor.matmul(
                                st[:, 0:384],
                                u01_r[:],
                                idneg2_r[:],
                                start=True,
                                stop=False,
                            )
                            premasked = True
                        else:
                            premasked = False
                        for (j, qoff, width, col) in chunks:
                            if is_diag and not premasked:
                                # write NEG*[q<k] into the first 128 cols,
                                # then accumulate the scores on top
                                nc.tensor.matmul(
                                    st[:, col : col + 128],
                                    u01_r[:],
                                    idneg_r[:],
                                    start=True,
                                    stop=False,
                                )
                            nc.tensor.matmul(
                                st[:, col : col + width],
                                kt[:, j * 128 : (j + 1) * 128],
                                qt[:, q0 + qoff : q0 + qoff + width],
                                start=not is_diag,
                                stop=True,
                            )
                        pt = ptp.tile([128, 1024], F32R, name="pt", tag="pt")
                        nc.scalar.activation(
                            pt[:, 0:extent],
                            st[:, 0:extent],
                            EXPF,
                            bias=zb[:],
                            scale=SCALE,
                        )
                        flush_pv()
                        if gi == 1:
                            flush_evac()

                        def pv(
                            chunks=chunks,
                            ot=ot,
                            lp=lp,
                            pt=pt,
                            vr=vr,
                            n_ch=n_ch,
                            is_last=(gi == len(groups) - 1),
                            b=b,
                            q0=q0,
                        ):
                            for (j, qoff, width, col) in chunks:
                                nc.tensor.matmul(
                                    ot[:, qoff : qoff + width],
                                    vr[:, j * 128 : (j + 1) * 128],
                                    pt[:, col : col + width],
                                    start=(j == 0),
                                    stop=(j == n_ch - 1),
                                )
                                nc.tensor.matmul(
                                    lp[:, qoff : qoff + width],
                                    ones_r[:],
                                    pt[:, col : col + width],
                                    start=(j == 0),
                                    stop=(j == n_ch - 1),
                                )
                            if not is_last:
                                return
                            # ---- evacuation (DVE): O^T * (1/l) ----
                            # lp rows are all equal (all-ones stationary).
                            recip = evacp.tile([128, 512], F32, name="recip")
                            nc.vector.reciprocal(recip[:], lp[:])
                            ots = evacp.tile([128, 512], F32, name="ots")
                            nc.vector.tensor_mul(ots[:], ot[:], recip[:])

                            def evac(b=b, q0=q0, ots=ots):
                                nc.sync.dma_start(
                                    o_d[b, :, q0 : q0 + 512], ots[:]
                                )

                            pending_evac[0] = evac

                        pending_pv[0] = pv

            flush_pv()
            flush_evac()
    nc.compile()
    return nc


_NC_CACHE = None


def _get_nc():
    global _NC_CACHE
    if _NC_CACHE is None:
        _NC_CACHE = build()
    return _NC_CACHE


def kernel(query, key, value, _trace=False):
    nc = _get_nc()
    in_maps = []
    for c in range(N_CORES):
        sl = slice(c * BPC, (c + 1) * BPC)
        in_maps.append(
            {
                "qt": np.ascontiguousarray(
                    np.asarray(query[sl], dtype=np.float32).transpose(0, 2, 1)
                ),
                "kt": np.ascontiguousarray(
                    np.asarray(key[sl], dtype=np.float32).transpose(0, 2, 1)
                ),
                "v": np.ascontiguousarray(value[sl], dtype=np.float32),
            }
        )
    res = run_bass_kernel_spmd(
        nc, in_maps, core_ids=list(range(N_CORES)), trace=_trace
    )
    out = np.concatenate(
        [res.results[c]["o"].transpose(0, 2, 1) for c in range(N_CORES)], axis=0
    )
    out = np.ascontiguousarray(out)
    if _trace:
        return out, res
    return out
